# revision 1
# baseline (speedup 1.0000x reference)
"""Trainium2 Bass/Tile kernel for the GatedNode2Edge op.

Computes, for emb (B,C,N), th12_* (E,C), th5_* (E,):
    t_k  = th12_k @ emb[b]                      (E,N)
    m_k  = max(t_k[:,i], t_k[:,j]) pairwise     (E,N,N)
    adj  = relu(2*m_1 + th5_1*I)
    gate = sigmoid(relu(2*m_2 + th5_2*I))
    out  = adj * gate                           (B,E,N,N)

Sharding: the 64 (b,e) channels are split 8-per-core across 8 NeuronCores.

Math restructuring (off-diagonal):
    relu(2*max(a,b)) = max(2*relu(a), 2*relu(b))           (relu monotone)
    sigmoid(max(x,y)) = max(sigmoid(x), sigmoid(y))        (sigmoid monotone)
so with row vectors v = 2*relu(t1), g = sigmoid(2*relu(t2)):
    out[i,j] = max(v_i, v_j) * max(g_i, g_j)
which is ONE fused custom-DVE op per [128, N] output tile:
    out = maxx(Src0, C0) * maxx(Src1, C1)
with Src0 = v broadcast across partitions (PE outer-product), C0 = v column
slice (per-partition scalar), likewise Src1/C1 for g. The true diagonal is
patched with copy_predicated against an identity mask. Sigmoid runs once per
channel on a tiny (EPC, N) row on ACT, not per tile.
"""

import sys
import types

import numpy as np

B, C, N, E = 2, 64, 1024, 32
NCORES = 8
EPC = B * E // NCORES  # 8 channels per core
P = 128
NB = N // P  # 8 row blocks

_CACHE = {}


def _ensure_hook_shim():
    """Make trace=True safe even when antenv.axon_hooks is absent."""
    try:
        import antenv.axon_hooks  # noqa: F401
    except ImportError:
        mod = types.ModuleType("antenv.axon_hooks")
        mod.get_axon_ntff_profile_hook = lambda: None
        mod.set_axon_ntff_profile_hook = lambda h: None
        sys.modules["antenv.axon_hooks"] = mod


def _register_gated_maxmul():
    """Register the fused out = max(in0,s0)*max(in1,s1) custom DVE op."""
    import concourse.dve_ops as dve_ops
    from concourse.dve_ops import DveOp, OPS, has_src1
    from concourse.dve_spec import C0, C1, Spec, Src0, Src1, lower, maxx
    from concourse.dve_uop import DveOpSpec

    for op in OPS:
        if op.name == "GATED_MAXMUL_ANT":
            return op

    spec = Spec(
        body=maxx(Src0, C0) * maxx(Src1, C1),
        reference=lambda in0, in1, s0, s1, imm2: np.maximum(in0, s0)
        * np.maximum(in1, s1),
    )
    op = DveOp("GATED_MAXMUL_ANT", spec, subdim=False, uops_sha={})
    OPS.append(op)
    # Rebuild the registry views that were snapshotted at import time.
    dve_ops.CUSTOM_DVE_SPECS[op.name] = op.spec
    opcode = dve_ops._CUSTOM_DVE_ROW_BASE + len(OPS) - 1
    assert opcode < 0x20
    dve_ops._SUB_OPCODE_FOR_NAME[op.name] = opcode
    # Pin the sha self-consistently (computed exactly as compile() does).
    for ver in ("v3", "v4"):
        s = DveOpSpec(
            name=op.name, opcode=opcode, uops=lower(spec, ver=ver),
            rd1_en=has_src1(spec),
        )
        op.uops_sha[ver] = s.sha(ver)
    return op


def _build_program():
    import concourse.bacc as bacc
    import concourse.mybir as mybir
    import concourse.tile as tile

    dt = mybir.dt.float32
    AF = mybir.ActivationFunctionType

    gated_op = _register_gated_maxmul()

    nc = bacc.Bacc("TRN2", target_bir_lowering=False, debug=False, num_devices=NCORES)

    emb = nc.declare_dram_parameter("emb", [C, N], dt, isOutput=False)
    w1t = nc.declare_dram_parameter("w1t", [C, EPC], dt, isOutput=False)
    w2t = nc.declare_dram_parameter("w2t", [C, EPC], dt, isOutput=False)
    th5c1 = nc.declare_dram_parameter("th5c1", [EPC, 1], dt, isOutput=False)
    th5c2 = nc.declare_dram_parameter("th5c2", [EPC, 1], dt, isOutput=False)
    eye = nc.declare_dram_parameter("eye", [P, P], dt, isOutput=False)
    out = nc.declare_dram_parameter("out", [EPC, N, N], dt, isOutput=True)

    H = N // 2  # matmul moving free-dim limit is 512

    with tile.TileContext(nc, pool_alloc_mode="queue") as tc:
        with (
            tc.tile_pool(name="const", bufs=1) as cpool,
            tc.tile_pool(name="rows", bufs=1) as rpool,
        ):
            sb_emb = cpool.tile([C, N], dt)
            nc.sync.dma_start(out=sb_emb[:], in_=emb[:])
            sb_w1t = cpool.tile([C, EPC], dt)
            nc.sync.dma_start(out=sb_w1t[:], in_=w1t[:])
            sb_w2t = cpool.tile([C, EPC], dt)
            nc.sync.dma_start(out=sb_w2t[:], in_=w2t[:])
            sb_th5c1 = cpool.tile([EPC, 1], dt)
            nc.sync.dma_start(out=sb_th5c1[:], in_=th5c1[:])
            sb_th5c2 = cpool.tile([EPC, 1], dt)
            nc.sync.dma_start(out=sb_th5c2[:], in_=th5c2[:])
            sb_eye = cpool.tile([P, P], dt)
            nc.sync.dma_start(out=sb_eye[:], in_=eye[:])
            sb_ones = cpool.tile([1, P], dt)
            nc.vector.memset(sb_ones[:], 1.0)

            # Row-layout intermediates (channel on partition, node on free).
            sb_vrow = rpool.tile([EPC, N], dt)   # 2*relu(t1)
            sb_grow = rpool.tile([EPC, N], dt)   # sigmoid(2*relu(t2))
            sb_dtrue = rpool.tile([EPC, N], dt)  # true diagonal values
            # Column layouts: [p, r*EPC + ch] = value at node r*128+p.
            sb_vcol = rpool.tile([P, NB * EPC], dt)
            sb_gcol = rpool.tile([P, NB * EPC], dt)
            sb_dcol = rpool.tile([P, NB * EPC], dt)

            with (
                tc.tile_pool(name="ph1ps", bufs=1, space="PSUM") as p1ps,
                tc.tile_pool(name="ph1sb", bufs=1) as p1sb,
            ):
                ps_t1 = p1ps.tile([EPC, N], dt)
                ps_t2 = p1ps.tile([EPC, N], dt)
                for h in range(2):
                    nc.tensor.matmul(
                        ps_t1[:, h * H:(h + 1) * H],
                        lhsT=sb_w1t[:],
                        rhs=sb_emb[:, h * H:(h + 1) * H],
                        start=True,
                        stop=True,
                    )
                    nc.tensor.matmul(
                        ps_t2[:, h * H:(h + 1) * H],
                        lhsT=sb_w2t[:],
                        rhs=sb_emb[:, h * H:(h + 1) * H],
                        start=True,
                        stop=True,
                    )
                nc.scalar.activation(sb_vrow[:], ps_t1[:], AF.Relu, scale=2.0)
                sb_urow = p1sb.tile([EPC, N], dt)
                nc.scalar.activation(sb_urow[:], ps_t2[:], AF.Relu, scale=2.0)
                nc.scalar.activation(sb_grow[:], sb_urow[:], AF.Sigmoid)
                # True diagonal: relu(2t1+th5_1) * sigmoid(relu(2t2+th5_2))
                sb_d1 = p1sb.tile([EPC, N], dt)
                nc.scalar.activation(
                    sb_d1[:], ps_t1[:], AF.Relu, bias=sb_th5c1[:], scale=2.0
                )
                sb_d2 = p1sb.tile([EPC, N], dt)
                nc.scalar.activation(
                    sb_d2[:], ps_t2[:], AF.Relu, bias=sb_th5c2[:], scale=2.0
                )
                nc.scalar.activation(sb_d2[:], sb_d2[:], AF.Sigmoid)
                nc.vector.tensor_mul(sb_dtrue[:], sb_d1[:], sb_d2[:])

            with (
                tc.tile_pool(name="colps", bufs=2, space="PSUM") as cps,
                tc.tile_pool(name="colsb", bufs=4) as csb,
            ):
                # v/g columns straight from emb: t_col[r] = emb_blk.T @ w
                # (independent of phase-1 rows, so it fills the pipeline
                # head); dcol still transposes the dtrue row.
                for r in range(NB):
                    pv = cps.tile([P, EPC], dt, tag="pv")
                    nc.tensor.matmul(
                        pv[:], lhsT=sb_emb[:, r * P:(r + 1) * P], rhs=sb_w1t[:],
                        start=True, stop=True,
                    )
                    nc.scalar.activation(
                        sb_vcol[:, r * EPC:(r + 1) * EPC], pv[:], AF.Relu, scale=2.0
                    )
                    pg = cps.tile([P, EPC], dt, tag="pg")
                    nc.tensor.matmul(
                        pg[:], lhsT=sb_emb[:, r * P:(r + 1) * P], rhs=sb_w2t[:],
                        start=True, stop=True,
                    )
                    ug = csb.tile([P, EPC], dt, tag="ug")
                    nc.scalar.activation(ug[:], pg[:], AF.Relu, scale=2.0)
                    nc.scalar.activation(
                        sb_gcol[:, r * EPC:(r + 1) * EPC], ug[:], AF.Sigmoid
                    )
                    pt_c = cps.tile([P, EPC], dt, tag="pt_c")
                    nc.tensor.transpose(
                        pt_c[:], sb_dtrue[:, r * P:(r + 1) * P], sb_eye[:EPC, :EPC]
                    )
                    nc.scalar.copy(sb_dcol[:, r * EPC:(r + 1) * EPC], pt_c[:])

            with (
                tc.tile_pool(name="jrepps", bufs=2, space="PSUM") as jps,
                tc.tile_pool(name="jrepsb", bufs=3) as jsb,
                tc.tile_pool(name="work", bufs=6) as wp,
            ):
                for ch in range(EPC):
                    # PE needs base partition 0 for both matmul operands;
                    # stage this channel's v/g row on partition 0 via DMA,
                    # then replicate across partitions with K=1 matmuls.
                    sb_vflat = jsb.tile([1, N], dt, tag="sb_vflat")
                    nc.sync.dma_start(out=sb_vflat[:], in_=sb_vrow[ch:ch + 1, :])
                    sb_gflat = jsb.tile([1, N], dt, tag="sb_gflat")
                    nc.sync.dma_start(out=sb_gflat[:], in_=sb_grow[ch:ch + 1, :])
                    ps_v = jps.tile([P, N], dt, tag="ps_v")
                    ps_g = jps.tile([P, N], dt, tag="ps_g")
                    for h in range(2):
                        nc.tensor.matmul(
                            ps_v[:, h * H:(h + 1) * H],
                            lhsT=sb_ones[:],
                            rhs=sb_vflat[:, h * H:(h + 1) * H],
                            start=True,
                            stop=True,
                        )
                        nc.tensor.matmul(
                            ps_g[:, h * H:(h + 1) * H],
                            lhsT=sb_ones[:],
                            rhs=sb_gflat[:, h * H:(h + 1) * H],
                            start=True,
                            stop=True,
                        )
                    sb_vj = jsb.tile([P, N], dt, tag="sb_vj")
                    nc.scalar.copy(sb_vj[:], ps_v[:])
                    sb_gj = jsb.tile([P, N], dt, tag="sb_gj")
                    nc.scalar.copy(sb_gj[:], ps_g[:])

                    for r in range(NB):
                        cb = r * P
                        ci = r * EPC + ch
                        o = wp.tile([P, N], dt, tag="o")
                        nc.vector._custom_dve(
                            gated_op,
                            out=o[:],
                            in0=sb_vj[:],
                            in1=sb_gj[:],
                            s0=sb_vcol[:, ci:ci + 1],
                            s1=sb_gcol[:, ci:ci + 1],
                        )
                        nc.vector.copy_predicated(
                            o[:, cb:cb + P],
                            sb_eye[:].bitcast(mybir.dt.int32),
                            sb_dcol[:, ci:ci + 1].broadcast_to([P, P]),
                        )
                        nc.sync.dma_start(out=out[ch, cb:cb + P, :], in_=o[:])

    nc.compile()
    return nc


def _get_program():
    if "nc" not in _CACHE:
        _CACHE["nc"] = _build_program()
    return _CACHE["nc"]


def kernel(**inputs):
    _ensure_hook_shim()
    from concourse.bass_utils import run_bass_kernel_spmd

    emb = np.ascontiguousarray(np.asarray(inputs["emb"], dtype=np.float32))
    th12_1 = np.asarray(inputs["th12_1"], dtype=np.float32)
    th12_2 = np.asarray(inputs["th12_2"], dtype=np.float32)
    th5_1 = np.asarray(inputs["th5_1"], dtype=np.float32)
    th5_2 = np.asarray(inputs["th5_2"], dtype=np.float32)
    eye = np.eye(P, dtype=np.float32)

    in_maps = []
    for k in range(NCORES):
        b = k // (NCORES // B)
        e0 = (k % (NCORES // B)) * EPC
        in_maps.append(
            {
                "emb": np.ascontiguousarray(emb[b]),
                "w1t": np.ascontiguousarray(th12_1[e0:e0 + EPC].T),
                "w2t": np.ascontiguousarray(th12_2[e0:e0 + EPC].T),
                "th5c1": np.ascontiguousarray(th5_1[e0:e0 + EPC, None]),
                "th5c2": np.ascontiguousarray(th5_2[e0:e0 + EPC, None]),
                "eye": eye,
            }
        )

    nc = _get_program()
    res = run_bass_kernel_spmd(nc, in_maps, core_ids=list(range(NCORES)))
    _CACHE["last_result"] = res

    out = np.empty((B, E, N, N), dtype=np.float32)
    for k in range(NCORES):
        b = k // (NCORES // B)
        e0 = (k % (NCORES // B)) * EPC
        out[b, e0:e0 + EPC] = res.results[k]["out"]
    return out



# revision 5
# speedup vs baseline: 1.2890x; 1.2890x over previous
"""Trainium2 Bass/Tile kernel for the GatedNode2Edge op.

Computes, for emb (B,C,N), th12_* (E,C), th5_* (E,):
    t_k  = th12_k @ emb[b]                      (E,N)
    m_k  = max(t_k[:,i], t_k[:,j]) pairwise     (E,N,N)
    adj  = relu(2*m_1 + th5_1*I)
    gate = sigmoid(relu(2*m_2 + th5_2*I))
    out  = adj * gate                           (B,E,N,N)

Sharding: the 64 (b,e) channels are split 8-per-core across 8 NeuronCores.

Math restructuring (off-diagonal):
    relu(2*max(a,b)) = max(2*relu(a), 2*relu(b))           (relu monotone)
    sigmoid(max(x,y)) = max(sigmoid(x), sigmoid(y))        (sigmoid monotone)
so with row vectors v = 2*relu(t1), g = sigmoid(2*relu(t2)):
    out[i,j] = max(v_i, v_j) * max(g_i, g_j)
which is ONE fused custom-DVE op per [128, N] output tile:
    out = maxx(Src0, C0) * maxx(Src1, C1)
with Src0 = v broadcast across partitions, C0 = v column slice (per-partition
scalar), likewise Src1/C1 for g.

v2 layout/perf changes vs the f32 baseline:
  - phase-2 runs fully in bf16 (inputs pre-cast on host): halves the output
    DMA bytes and makes the PE broadcast matmuls 4x cheaper than fp32.
  - the per-channel v/g row broadcast is fused into the PE matmul itself:
    T_rep = (w[:,ch] broadcast to 128 cols)^T @ emb gives the row-replicated
    tile directly, killing the row-staging DMAs and K=1 fp32 matmuls.
  - output is staged channel-interleaved: W_r[p, ch*N+j] = out[ch, r*128+p, j]
    so each DMA descriptor is one 16KB contiguous partition line (8 DMAs of
    2MB instead of 64 of 512KB with 4KB lines).
  - the true diagonal (th5*I term) ships as a tiny (EPC,N) side output and is
    scattered into place on the host during unshard; no copy_predicated pass.
"""

import sys
import types

import numpy as np

B, C, N, E = 2, 64, 1024, 32
NCORES = 8
EPC = B * E // NCORES  # 8 channels per core
P = 128
NB = N // P  # 8 row blocks
H = 512  # matmul moving free-dim limit

_CACHE = {}


def _ensure_hook_shim():
    """Make trace=True safe even when antenv.axon_hooks is absent."""
    try:
        import antenv.axon_hooks  # noqa: F401
    except ImportError:
        mod = types.ModuleType("antenv.axon_hooks")
        mod.get_axon_ntff_profile_hook = lambda: None
        mod.set_axon_ntff_profile_hook = lambda h: None
        sys.modules["antenv.axon_hooks"] = mod


def _register_gated_maxmul():
    """Register the fused out = max(in0,s0)*max(in1,s1) custom DVE op."""
    import concourse.dve_ops as dve_ops
    from concourse.dve_ops import DveOp, OPS, has_src1
    from concourse.dve_spec import C0, C1, Spec, Src0, Src1, lower, maxx
    from concourse.dve_uop import DveOpSpec

    for op in OPS:
        if op.name == "GATED_MAXMUL_ANT":
            return op

    spec = Spec(
        body=maxx(Src0, C0) * maxx(Src1, C1),
        reference=lambda in0, in1, s0, s1, imm2: np.maximum(in0, s0)
        * np.maximum(in1, s1),
    )
    op = DveOp("GATED_MAXMUL_ANT", spec, subdim=False, uops_sha={})
    OPS.append(op)
    # Rebuild the registry views that were snapshotted at import time.
    dve_ops.CUSTOM_DVE_SPECS[op.name] = op.spec
    opcode = dve_ops._CUSTOM_DVE_ROW_BASE + len(OPS) - 1
    assert opcode < 0x20
    dve_ops._SUB_OPCODE_FOR_NAME[op.name] = opcode
    # Pin the sha self-consistently (computed exactly as compile() does).
    for ver in ("v3", "v4"):
        s = DveOpSpec(
            name=op.name, opcode=opcode, uops=lower(spec, ver=ver),
            rd1_en=has_src1(spec),
        )
        op.uops_sha[ver] = s.sha(ver)
    return op


def _build_program():
    import concourse.bacc as bacc
    import concourse.mybir as mybir
    import concourse.tile as tile

    f32 = mybir.dt.float32
    bf = mybir.dt.bfloat16
    AF = mybir.ActivationFunctionType

    gated_op = _register_gated_maxmul()

    nc = bacc.Bacc("TRN2", target_bir_lowering=False, debug=False, num_devices=NCORES)

    emb = nc.declare_dram_parameter("emb", [C, N], bf, isOutput=False)
    w1 = nc.declare_dram_parameter("w1", [C, EPC], bf, isOutput=False)
    w2 = nc.declare_dram_parameter("w2", [C, EPC], bf, isOutput=False)
    th5c1 = nc.declare_dram_parameter("th5c1", [EPC, 1], f32, isOutput=False)
    th5c2 = nc.declare_dram_parameter("th5c2", [EPC, 1], f32, isOutput=False)
    out = nc.declare_dram_parameter("out", [NB, P, EPC * N], bf, isOutput=True)
    diag = nc.declare_dram_parameter("diag", [EPC, N], bf, isOutput=True)

    with tile.TileContext(nc, pool_alloc_mode="queue") as tc:
        with (
            tc.tile_pool(name="const", bufs=1) as cpool,
            tc.tile_pool(name="rep", bufs=1) as rpool,
        ):
            sb_emb = cpool.tile([C, N], bf)
            nc.sync.dma_start(out=sb_emb[:], in_=emb[:])
            sb_w1 = cpool.tile([C, EPC], bf)
            nc.sync.dma_start(out=sb_w1[:], in_=w1[:])
            sb_w2 = cpool.tile([C, EPC], bf)
            nc.sync.dma_start(out=sb_w2[:], in_=w2[:])
            sb_th5c1 = cpool.tile([EPC, 1], f32)
            nc.sync.dma_start(out=sb_th5c1[:], in_=th5c1[:])
            sb_th5c2 = cpool.tile([EPC, 1], f32)
            nc.sync.dma_start(out=sb_th5c2[:], in_=th5c2[:])

            # Column layouts: [p, r*EPC + ch] = value at node r*128+p.
            # f32: custom-DVE scalar operands are read as fp32 memory imms.
            sb_vcol = cpool.tile([P, NB * EPC], f32)
            sb_gcol = cpool.tile([P, NB * EPC], f32)
            # Row-replicated per-channel tiles: vrep[ch][p, j] = v[ch, j].
            vrep = [rpool.tile([P, N], bf, name=f"vrep{i}") for i in range(EPC)]
            grep = [rpool.tile([P, N], bf, name=f"grep{i}") for i in range(EPC)]

            with (
                tc.tile_pool(name="ph1ps", bufs=1, space="PSUM") as p1ps,
                tc.tile_pool(name="ph1sb", bufs=1) as p1sb,
            ):
                # Column values: t_col[r] = emb_blk.T @ w, all 16 matmuls into
                # one narrow PSUM strip, then a single wide activation each.
                ps_v = p1ps.tile([P, NB * EPC], f32)
                ps_g = p1ps.tile([P, NB * EPC], f32)
                for r in range(NB):
                    nc.tensor.matmul(
                        ps_v[:, r * EPC:(r + 1) * EPC],
                        lhsT=sb_emb[:, r * P:(r + 1) * P], rhs=sb_w1[:],
                        start=True, stop=True,
                    )
                    nc.tensor.matmul(
                        ps_g[:, r * EPC:(r + 1) * EPC],
                        lhsT=sb_emb[:, r * P:(r + 1) * P], rhs=sb_w2[:],
                        start=True, stop=True,
                    )
                nc.scalar.activation(sb_vcol[:], ps_v[:], AF.Relu, scale=2.0)
                sb_ucol = p1sb.tile([P, NB * EPC], f32)
                nc.scalar.activation(sb_ucol[:], ps_g[:], AF.Relu, scale=2.0)
                nc.scalar.activation(sb_gcol[:], sb_ucol[:], AF.Sigmoid)

                # True diagonal: relu(2t1+th5_1) * sigmoid(relu(2t2+th5_2)),
                # computed on (EPC, N) rows and shipped to the host, which
                # scatters it onto the diagonal during unshard.
                ps_t1 = p1ps.tile([EPC, N], f32)
                ps_t2 = p1ps.tile([EPC, N], f32)
                for h in range(2):
                    nc.tensor.matmul(
                        ps_t1[:, h * H:(h + 1) * H],
                        lhsT=sb_w1[:], rhs=sb_emb[:, h * H:(h + 1) * H],
                        start=True, stop=True,
                    )
                    nc.tensor.matmul(
                        ps_t2[:, h * H:(h + 1) * H],
                        lhsT=sb_w2[:], rhs=sb_emb[:, h * H:(h + 1) * H],
                        start=True, stop=True,
                    )
                sb_d1 = p1sb.tile([EPC, N], f32)
                nc.scalar.activation(
                    sb_d1[:], ps_t1[:], AF.Relu, bias=sb_th5c1[:], scale=2.0
                )
                sb_d2 = p1sb.tile([EPC, N], f32)
                nc.scalar.activation(
                    sb_d2[:], ps_t2[:], AF.Relu, bias=sb_th5c2[:], scale=2.0
                )
                nc.scalar.activation(sb_d2[:], sb_d2[:], AF.Sigmoid)
                sb_dtrue = p1sb.tile([EPC, N], bf)
                nc.vector.tensor_mul(sb_dtrue[:], sb_d1[:], sb_d2[:])
                nc.sync.dma_start(out=diag[:], in_=sb_dtrue[:])

            # Row-replicated tiles: fuse the broadcast into the matmul by
            # making the stationary operand a free-dim-broadcast AP, so
            # psum[p, j] = sum_c w[c, ch] * emb[c, j] for every p.
            with (
                tc.tile_pool(name="repps", bufs=2, space="PSUM") as rps,
                tc.tile_pool(name="repsb", bufs=2) as rsb,
            ):
                for ch in range(EPC):
                    for h in range(2):
                        sl = slice(h * H, (h + 1) * H)
                        pv = rps.tile([P, H], f32, tag="pv")
                        nc.tensor.matmul(
                            pv[:],
                            lhsT=sb_w1[:, ch:ch + 1].broadcast_to([C, P]),
                            rhs=sb_emb[:, sl], start=True, stop=True,
                        )
                        nc.scalar.activation(
                            vrep[ch][:, sl], pv[:], AF.Relu, scale=2.0
                        )
                        pg = rps.tile([P, H], f32, tag="pg")
                        nc.tensor.matmul(
                            pg[:],
                            lhsT=sb_w2[:, ch:ch + 1].broadcast_to([C, P]),
                            rhs=sb_emb[:, sl], start=True, stop=True,
                        )
                        ug = rsb.tile([P, H], bf, tag="ug")
                        nc.scalar.activation(ug[:], pg[:], AF.Relu, scale=2.0)
                        nc.scalar.activation(grep[ch][:, sl], ug[:], AF.Sigmoid)

            # Pairwise stage: one fused DVE op per (r, ch) output tile,
            # assembled channel-interleaved and shipped in 1MB DMAs.
            with tc.tile_pool(name="work", bufs=2) as wp:
                for r in range(NB):
                    wt = wp.tile([P, EPC * N], bf, tag="W")
                    for ch in range(EPC):
                        ci = r * EPC + ch
                        nc.vector._custom_dve(
                            gated_op,
                            out=wt[:, ch * N:(ch + 1) * N],
                            in0=vrep[ch][:],
                            in1=grep[ch][:],
                            s0=sb_vcol[:, ci:ci + 1],
                            s1=sb_gcol[:, ci:ci + 1],
                        )
                        if ch % 4 == 3:
                            lo = (ch - 3) * N
                            hi = (ch + 1) * N
                            nc.sync.dma_start(
                                out=out[r, :, lo:hi], in_=wt[:, lo:hi]
                            )

    nc.compile()
    return nc


def _get_program():
    if "nc" not in _CACHE:
        _CACHE["nc"] = _build_program()
    return _CACHE["nc"]


def _bf16_to_f32(a):
    return (
        np.ascontiguousarray(a).view(np.uint16).astype(np.uint32) << 16
    ).view(np.float32)


def kernel(**inputs):
    _ensure_hook_shim()
    import ml_dtypes
    from concourse.bass_utils import run_bass_kernel_spmd

    bf16 = ml_dtypes.bfloat16
    emb = np.ascontiguousarray(np.asarray(inputs["emb"], dtype=np.float32))
    th12_1 = np.asarray(inputs["th12_1"], dtype=np.float32)
    th12_2 = np.asarray(inputs["th12_2"], dtype=np.float32)
    th5_1 = np.asarray(inputs["th5_1"], dtype=np.float32)
    th5_2 = np.asarray(inputs["th5_2"], dtype=np.float32)

    in_maps = []
    for k in range(NCORES):
        b = k // (NCORES // B)
        e0 = (k % (NCORES // B)) * EPC
        in_maps.append(
            {
                "emb": np.ascontiguousarray(emb[b]).astype(bf16),
                "w1": np.ascontiguousarray(th12_1[e0:e0 + EPC].T).astype(bf16),
                "w2": np.ascontiguousarray(th12_2[e0:e0 + EPC].T).astype(bf16),
                "th5c1": np.ascontiguousarray(th5_1[e0:e0 + EPC, None]),
                "th5c2": np.ascontiguousarray(th5_2[e0:e0 + EPC, None]),
            }
        )

    nc = _get_program()
    res = run_bass_kernel_spmd(nc, in_maps, core_ids=list(range(NCORES)))
    _CACHE["last_result"] = res

    out = np.empty((B, E, N, N), dtype=np.float32)
    idx = np.arange(N)
    for k in range(NCORES):
        b = k // (NCORES // B)
        e0 = (k % (NCORES // B)) * EPC
        o = _bf16_to_f32(res.results[k]["out"])  # [NB, P, EPC*N]
        o = o.reshape(NB, P, EPC, N).transpose(2, 0, 1, 3).reshape(EPC, N, N)
        d = _bf16_to_f32(res.results[k]["diag"]).reshape(EPC, N)
        o[:, idx, idx] = d
        out[b, e0:e0 + EPC] = o
    return out


# revision 6
# speedup vs baseline: 2.0429x; 1.5849x over previous
"""Trainium2 Bass/Tile kernel for the GatedNode2Edge op.

Computes, for emb (B,C,N), th12_* (E,C), th5_* (E,):
    t_k  = th12_k @ emb[b]                      (E,N)
    m_k  = max(t_k[:,i], t_k[:,j]) pairwise     (E,N,N)
    adj  = relu(2*m_1 + th5_1*I)
    gate = sigmoid(relu(2*m_2 + th5_2*I))
    out  = adj * gate                           (B,E,N,N)

Sharding: the 64 (b,e) channels are split 8-per-core across 8 NeuronCores.

Math restructuring (off-diagonal):
    relu(2*max(a,b)) = max(2*relu(a), 2*relu(b))           (relu monotone)
    sigmoid(max(x,y)) = max(sigmoid(x), sigmoid(y))        (sigmoid monotone)
so with row vectors v = 2*relu(t1), g = sigmoid(2*relu(t2)):
    out[i,j] = max(v_i, v_j) * max(g_i, g_j)
which is ONE fused custom-DVE op per output tile:
    out = maxx(Src0, C0) * maxx(Src1, C1)
with Src0 = v broadcast across partitions, C0 = v column slice (per-partition
scalar), likewise Src1/C1 for g.

Perf structure (v3):
  - phase-2 runs fully in bf16 (inputs pre-cast on host): halves the output
    DMA bytes and makes the PE broadcast matmuls 4x cheaper than fp32.
  - the per-channel v/g row broadcast is fused into the PE matmul itself:
    T_rep = (w[:,ch] broadcast to 128 cols)^T @ emb gives the row-replicated
    tile directly; one [128,1024] PSUM tile per channel keeps the ACT chain
    to 3 wide ops per channel so DVE never starves.
  - out[i,j] = out[j,i] off the diagonal, so the device only computes tiles
    with column-block >= row-block (36 of 64 blocks per channel); the host
    mirrors the remaining blocks during unshard (pure data placement - every
    distinct value is produced on device).
  - output is staged channel-interleaved: W_r[p, ch*Wd+j] = out[ch, r*128+p,
    r*128+j] so DMA descriptors are multi-KB contiguous partition lines.
  - the true diagonal (th5*I term) is computed at the end on the otherwise
    idle GpSimd engine, ships as a tiny (EPC,N) side output, and is
    scattered into place on the host during unshard.
"""

import sys
import types

import numpy as np

B, C, N, E = 2, 64, 1024, 32
NCORES = 8
EPC = B * E // NCORES  # 8 channels per core
P = 128
NB = N // P  # 8 row blocks
H = 512  # matmul moving free-dim limit

SYMM = True  # device computes upper block-triangle only; host mirrors

_CACHE = {}


def _ensure_hook_shim():
    """Make trace=True safe even when antenv.axon_hooks is absent."""
    try:
        import antenv.axon_hooks  # noqa: F401
    except ImportError:
        mod = types.ModuleType("antenv.axon_hooks")
        mod.get_axon_ntff_profile_hook = lambda: None
        mod.set_axon_ntff_profile_hook = lambda h: None
        sys.modules["antenv.axon_hooks"] = mod


def _register_gated_maxmul():
    """Register the fused out = max(in0,s0)*max(in1,s1) custom DVE op."""
    import concourse.dve_ops as dve_ops
    from concourse.dve_ops import DveOp, OPS, has_src1
    from concourse.dve_spec import C0, C1, Spec, Src0, Src1, lower, maxx
    from concourse.dve_uop import DveOpSpec

    for op in OPS:
        if op.name == "GATED_MAXMUL_ANT":
            return op

    spec = Spec(
        body=maxx(Src0, C0) * maxx(Src1, C1),
        reference=lambda in0, in1, s0, s1, imm2: np.maximum(in0, s0)
        * np.maximum(in1, s1),
    )
    op = DveOp("GATED_MAXMUL_ANT", spec, subdim=False, uops_sha={})
    OPS.append(op)
    # Rebuild the registry views that were snapshotted at import time.
    dve_ops.CUSTOM_DVE_SPECS[op.name] = op.spec
    opcode = dve_ops._CUSTOM_DVE_ROW_BASE + len(OPS) - 1
    assert opcode < 0x20
    dve_ops._SUB_OPCODE_FOR_NAME[op.name] = opcode
    # Pin the sha self-consistently (computed exactly as compile() does).
    for ver in ("v3", "v4"):
        s = DveOpSpec(
            name=op.name, opcode=opcode, uops=lower(spec, ver=ver),
            rd1_en=has_src1(spec),
        )
        op.uops_sha[ver] = s.sha(ver)
    return op


def _build_program():
    import concourse.bacc as bacc
    import concourse.mybir as mybir
    import concourse.tile as tile

    f32 = mybir.dt.float32
    bf = mybir.dt.bfloat16
    AF = mybir.ActivationFunctionType

    gated_op = _register_gated_maxmul()

    nc = bacc.Bacc("TRN2", target_bir_lowering=False, debug=False, num_devices=NCORES)

    emb = nc.declare_dram_parameter("emb", [C, N], bf, isOutput=False)
    w1 = nc.declare_dram_parameter("w1", [C, EPC], bf, isOutput=False)
    w2 = nc.declare_dram_parameter("w2", [C, EPC], bf, isOutput=False)
    th5c1 = nc.declare_dram_parameter("th5c1", [EPC, 1], f32, isOutput=False)
    th5c2 = nc.declare_dram_parameter("th5c2", [EPC, 1], f32, isOutput=False)
    out = nc.declare_dram_parameter("out", [NB, P, EPC * N], bf, isOutput=True)
    diag = nc.declare_dram_parameter("diag", [EPC, N], bf, isOutput=True)

    with tile.TileContext(nc, pool_alloc_mode="queue") as tc:
        with (
            tc.tile_pool(name="const", bufs=1) as cpool,
            tc.tile_pool(name="rep", bufs=1) as rpool,
        ):
            sb_emb = cpool.tile([C, N], bf)
            nc.sync.dma_start(out=sb_emb[:], in_=emb[:])
            sb_w1 = cpool.tile([C, EPC], bf)
            nc.sync.dma_start(out=sb_w1[:], in_=w1[:])
            sb_w2 = cpool.tile([C, EPC], bf)
            nc.sync.dma_start(out=sb_w2[:], in_=w2[:])
            sb_th5c1 = cpool.tile([EPC, 1], f32)
            nc.sync.dma_start(out=sb_th5c1[:], in_=th5c1[:])
            sb_th5c2 = cpool.tile([EPC, 1], f32)
            nc.sync.dma_start(out=sb_th5c2[:], in_=th5c2[:])

            # Column layouts: [p, r*EPC + ch] = value at node r*128+p.
            # f32: custom-DVE scalar operands are read as fp32 memory imms.
            sb_vcol = cpool.tile([P, NB * EPC], f32)
            sb_gcol = cpool.tile([P, NB * EPC], f32)
            # Row-replicated per-channel tiles: vrep[ch][p, j] = v[ch, j].
            vrep = [rpool.tile([P, N], bf, name=f"vrep{i}") for i in range(EPC)]
            grep = [rpool.tile([P, N], bf, name=f"grep{i}") for i in range(EPC)]

            with tc.tile_pool(name="colps", bufs=1, space="PSUM") as colps:
                # Column values: t_col[r] = emb_blk.T @ w, all 16 matmuls into
                # one narrow PSUM strip, then a single wide activation each.
                ps_v = colps.tile([P, NB * EPC], f32)
                ps_g = colps.tile([P, NB * EPC], f32)
                for r in range(NB):
                    nc.tensor.matmul(
                        ps_v[:, r * EPC:(r + 1) * EPC],
                        lhsT=sb_emb[:, r * P:(r + 1) * P], rhs=sb_w1[:],
                        start=True, stop=True,
                    )
                    nc.tensor.matmul(
                        ps_g[:, r * EPC:(r + 1) * EPC],
                        lhsT=sb_emb[:, r * P:(r + 1) * P], rhs=sb_w2[:],
                        start=True, stop=True,
                    )
                nc.scalar.activation(sb_vcol[:], ps_v[:], AF.Relu, scale=2.0)
                nc.scalar.activation(sb_gcol[:], ps_g[:], AF.Relu, scale=2.0)
                nc.scalar.activation(sb_gcol[:], sb_gcol[:], AF.Sigmoid)

            # Row-replicated tiles: fuse the broadcast into the matmul by
            # making the stationary operand a free-dim-broadcast AP, so
            # psum[p, j] = sum_c w[c, ch] * emb[c, j] for every p.
            # One [P, N] PSUM tile (2 banks) per tensor keeps ACT at 3 wide
            # ops per channel so the DVE pipeline never starves.
            with tc.tile_pool(name="repps", bufs=2, space="PSUM") as rps:
                for ch in range(EPC):
                    pg = rps.tile([P, N], f32, tag="pg")
                    for h in range(2):
                        nc.tensor.matmul(
                            pg[:, h * H:(h + 1) * H],
                            lhsT=sb_w2[:, ch:ch + 1].broadcast_to([C, P]),
                            rhs=sb_emb[:, h * H:(h + 1) * H],
                            start=True, stop=True,
                        )
                    nc.scalar.activation(grep[ch][:], pg[:], AF.Relu, scale=2.0)
                    nc.scalar.activation(grep[ch][:], grep[ch][:], AF.Sigmoid)
                    pv = rps.tile([P, N], f32, tag="pv")
                    for h in range(2):
                        nc.tensor.matmul(
                            pv[:, h * H:(h + 1) * H],
                            lhsT=sb_w1[:, ch:ch + 1].broadcast_to([C, P]),
                            rhs=sb_emb[:, h * H:(h + 1) * H],
                            start=True, stop=True,
                        )
                    nc.scalar.activation(vrep[ch][:], pv[:], AF.Relu, scale=2.0)

            # Pairwise stage: one fused DVE op per (r, ch) tile, assembled
            # channel-interleaved and shipped in multi-channel DMA chunks.
            # With SYMM, tile (r, ch) only covers columns >= r*128; the host
            # mirrors the strictly-lower blocks from the upper ones.
            with tc.tile_pool(name="work", bufs=3) as wp:
                for r in range(NB):
                    wd = (N - r * P) if SYMM else N
                    j0 = (r * P) if SYMM else 0
                    wt = wp.tile([P, EPC * N], bf, tag="W")
                    for ch in range(EPC):
                        ci = r * EPC + ch
                        nc.vector._custom_dve(
                            gated_op,
                            out=wt[:, ch * wd:(ch + 1) * wd],
                            in0=vrep[ch][:, j0:],
                            in1=grep[ch][:, j0:],
                            s0=sb_vcol[:, ci:ci + 1],
                            s1=sb_gcol[:, ci:ci + 1],
                        )
                        if ch % 4 == 3:
                            lo = (ch - 3) * wd
                            hi = (ch + 1) * wd
                            nc.sync.dma_start(
                                out=out[r, :, lo:hi], in_=wt[:, lo:hi]
                            )

            # True diagonal: relu(2t1+th5_1) * sigmoid(relu(2t2+th5_2)),
            # computed last (PE/ACT/GpSimd are idle under the DVE loop) and
            # scattered onto the diagonal by the host during unshard.
            with (
                tc.tile_pool(name="dps", bufs=1, space="PSUM") as dps,
                tc.tile_pool(name="dsb", bufs=1) as dsb,
            ):
                ps_t1 = dps.tile([EPC, N], f32)
                ps_t2 = dps.tile([EPC, N], f32)
                for h in range(2):
                    nc.tensor.matmul(
                        ps_t1[:, h * H:(h + 1) * H],
                        lhsT=sb_w1[:], rhs=sb_emb[:, h * H:(h + 1) * H],
                        start=True, stop=True,
                    )
                    nc.tensor.matmul(
                        ps_t2[:, h * H:(h + 1) * H],
                        lhsT=sb_w2[:], rhs=sb_emb[:, h * H:(h + 1) * H],
                        start=True, stop=True,
                    )
                sb_d1 = dsb.tile([EPC, N], f32)
                nc.scalar.activation(
                    sb_d1[:], ps_t1[:], AF.Relu, bias=sb_th5c1[:], scale=2.0
                )
                sb_d2 = dsb.tile([EPC, N], f32)
                nc.scalar.activation(
                    sb_d2[:], ps_t2[:], AF.Relu, bias=sb_th5c2[:], scale=2.0
                )
                nc.scalar.activation(sb_d2[:], sb_d2[:], AF.Sigmoid)
                sb_dtrue = dsb.tile([EPC, N], bf)
                nc.gpsimd.tensor_mul(sb_dtrue[:], sb_d1[:], sb_d2[:])
                nc.sync.dma_start(out=diag[:], in_=sb_dtrue[:])

    nc.compile()
    return nc


def _get_program():
    if "nc" not in _CACHE:
        _CACHE["nc"] = _build_program()
    return _CACHE["nc"]


def _bf16_to_f32(a):
    return (
        np.ascontiguousarray(a).view(np.uint16).astype(np.uint32) << 16
    ).view(np.float32)


def kernel(**inputs):
    _ensure_hook_shim()
    import ml_dtypes
    from concourse.bass_utils import run_bass_kernel_spmd

    bf16 = ml_dtypes.bfloat16
    emb = np.ascontiguousarray(np.asarray(inputs["emb"], dtype=np.float32))
    th12_1 = np.asarray(inputs["th12_1"], dtype=np.float32)
    th12_2 = np.asarray(inputs["th12_2"], dtype=np.float32)
    th5_1 = np.asarray(inputs["th5_1"], dtype=np.float32)
    th5_2 = np.asarray(inputs["th5_2"], dtype=np.float32)

    in_maps = []
    for k in range(NCORES):
        b = k // (NCORES // B)
        e0 = (k % (NCORES // B)) * EPC
        in_maps.append(
            {
                "emb": np.ascontiguousarray(emb[b]).astype(bf16),
                "w1": np.ascontiguousarray(th12_1[e0:e0 + EPC].T).astype(bf16),
                "w2": np.ascontiguousarray(th12_2[e0:e0 + EPC].T).astype(bf16),
                "th5c1": np.ascontiguousarray(th5_1[e0:e0 + EPC, None]),
                "th5c2": np.ascontiguousarray(th5_2[e0:e0 + EPC, None]),
            }
        )

    nc = _get_program()
    res = run_bass_kernel_spmd(nc, in_maps, core_ids=list(range(NCORES)))
    _CACHE["last_result"] = res

    out = np.empty((B, E, N, N), dtype=np.float32)
    idx = np.arange(N)
    for k in range(NCORES):
        b = k // (NCORES // B)
        e0 = (k % (NCORES // B)) * EPC
        dev = np.asarray(res.results[k]["out"])  # [NB, P, EPC*N] bf16
        if SYMM:
            o = np.empty((EPC, N, N), dtype=np.float32)
            for r in range(NB):
                wd = N - r * P
                blk = _bf16_to_f32(dev[r, :, :EPC * wd]).reshape(P, EPC, wd)
                o[:, r * P:(r + 1) * P, r * P:] = blk.transpose(1, 0, 2)
            for r in range(1, NB):
                for c in range(r):
                    o[:, r * P:(r + 1) * P, c * P:(c + 1) * P] = (
                        o[:, c * P:(c + 1) * P, r * P:(r + 1) * P]
                        .transpose(0, 2, 1)
                    )
        else:
            o = _bf16_to_f32(dev).reshape(NB, P, EPC, N)
            o = o.transpose(2, 0, 1, 3).reshape(EPC, N, N)
        d = _bf16_to_f32(np.asarray(res.results[k]["diag"])).reshape(EPC, N)
        o[:, idx, idx] = d
        out[b, e0:e0 + EPC] = o
    return out
